# revision 9
# baseline (speedup 1.0000x reference)
"""Fused GNN message-passing kernel for Trainium2, SPMD across 8 NeuronCores.

Reference math (h = concat(x_pred, x_prey, x_obst) per batch, N=1024, C=64, O=2C=128):
    d[i,j,:] = h[i] - h[j]
    e        = elu(d @ conv1_w.T + conv1_b)          # [N, N, O]
    W[i,j]   = e[i,j] @ conv2_w[0] + conv2_b        # [N, N]
    agg      = W @ h
    out      = h + elu(agg @ lin_w.T + lin_b)

Key identities used on-chip (conv1 is linear, so the N x N x O tensor is never
materialized in HBM):
    a[i,o]   = sum_c h[i,c] conv1_w[o,c]        s[i,o] = a[i,o] + conv1_b[o]
    x_ijo    = s_i[o] - a_j[o]
    elu(x)   = relu(x) + min(exp(x), 1) - 1
    exp(x_ijo) = exp(s_i[o]) * exp(-a_j[o]) = P[o,i] * Q[o,j]   (rank-1)

Per core: a 128-row block of W / output rows. Inner loop over i (128 iters x 2
batches): two dual-op DVE tensor_scalar passes build r = relu(x) and
m = min(exp(x),1) as [O=128, N=1024] bf16 tiles; four accumulating PE matmuls
against a shifted band matrix (c2 in column i) reduce over o directly into a
[128, 512] PSUM W block. The "-1" of the elu and conv2_b fold into one scalar
added when W leaves PSUM.
"""

import sys

if "/opt/trn_rl_repo" not in sys.path:
    sys.path.insert(0, "/opt/trn_rl_repo")

import numpy as np

import concourse.bass as bass
import concourse.bacc as bacc
import concourse.mybir as mybir
import concourse.tile as tile
from concourse.bass_utils import run_bass_kernel_spmd

F32 = mybir.dt.float32
BF16 = mybir.dt.bfloat16
AF = mybir.ActivationFunctionType
OP = mybir.AluOpType

B = 2          # batch
N = 1024       # total nodes (256 + 512 + 256)
C = 64         # hidden
O = 2 * C      # conv1 out channels = 128
NP_, NY, NO = 256, 512, 256
NCORES = 8
RPC = N // NCORES  # 128 output rows per core

_NC_CACHE = {}


def _build_nc(n_repeat=1):
    nc = bacc.Bacc(None, target_bir_lowering=False, debug=False)

    hA = nc.dram_tensor("hA", [B, N, C], F32, kind="ExternalInput")
    hM = nc.dram_tensor("hM", [B, RPC, C], F32, kind="ExternalInput")
    hAt = nc.dram_tensor("hAt", [B, C + 1, N], F32, kind="ExternalInput")
    hMt = nc.dram_tensor("hMt", [B, C + 1, RPC], F32, kind="ExternalInput")
    w1t = nc.dram_tensor("w1t", [C + 1, O], F32, kind="ExternalInput")
    band = nc.dram_tensor("band", [O, 2 * RPC], F32, kind="ExternalInput")
    lwt = nc.dram_tensor("lwt", [C + 1, C], F32, kind="ExternalInput")
    eye = nc.dram_tensor("eye", [128, 128], F32, kind="ExternalInput")
    wb = nc.dram_tensor("wb", [128, 1], F32, kind="ExternalInput")
    out = nc.dram_tensor("out", [B, RPC, C], F32, kind="ExternalOutput")

    with tile.TileContext(nc) as tc, \
         tc.tile_pool(name="const", bufs=1) as constp, \
         tc.tile_pool(name="setup", bufs=2) as setupp, \
         tc.tile_pool(name="stream", bufs=6) as streamp, \
         tc.tile_pool(name="pbig", bufs=4, space="PSUM") as pbig, \
         tc.tile_pool(name="psmall", bufs=3, space="PSUM") as psmall:

        # ---- constants (loaded once) ----
        w1t_sb = constp.tile([C + 1, O], F32, tag="w1t")
        nc.sync.dma_start(w1t_sb[:], w1t[:])
        band_f = constp.tile([O, 2 * RPC], F32, tag="bandf")
        nc.sync.dma_start(band_f[:], band[:])
        band_sb = constp.tile([O, 2 * RPC], BF16, tag="band")
        nc.vector.tensor_copy(band_sb[:], band_f[:])
        lwt_f = constp.tile([C + 1, C], F32, tag="lwtf")
        nc.sync.dma_start(lwt_f[:], lwt[:])
        lwt_sb = constp.tile([C + 1, C], BF16, tag="lwt")
        nc.vector.tensor_copy(lwt_sb[:], lwt_f[:])
        eye_f = constp.tile([128, 128], F32, tag="eyef")
        nc.sync.dma_start(eye_f[:], eye[:])
        eye_sb = constp.tile([128, 128], BF16, tag="eye")
        nc.vector.tensor_copy(eye_sb[:], eye_f[:])
        wb_sb = constp.tile([128, 1], F32, tag="wb")
        nc.sync.dma_start(wb_sb[:], wb[:])

        for b in [b for _ in range(n_repeat) for b in range(B)]:
            # ---- per-batch setup ----
            hAt_sb = setupp.tile([C + 1, N], F32, tag="hAt")
            nc.sync.dma_start(hAt_sb[:], hAt[b])
            hMt_sb = setupp.tile([C + 1, RPC], F32, tag="hMt")
            nc.sync.dma_start(hMt_sb[:], hMt[b])
            hA_sb = setupp.tile([128, 8, C], F32, tag="hAsb")
            nc.sync.dma_start(hA_sb[:], hA[b].rearrange("(t p) c -> p t c", p=128))
            hM_sb = setupp.tile([RPC, C], F32, tag="hMsb")
            nc.sync.dma_start(hM_sb[:], hM[b])
            hA_bf = setupp.tile([128, 8, C], BF16, tag="hAbf")
            nc.vector.tensor_copy(hA_bf[:], hA_sb[:])

            # a^T[o, j] (no bias) in two 512-wide chunks; s[o, i_mine] with bias
            a_ps = [pbig.tile([O, 512], F32, tag="big", name=f"a_ps{ch}")
                    for ch in range(2)]
            for ch in range(2):
                nc.tensor.matmul(
                    a_ps[ch][:], w1t_sb[0:C, :],
                    hAt_sb[0:C, 512 * ch:512 * (ch + 1)],
                    start=True, stop=True,
                )
            s_ps = psmall.tile([O, RPC], F32, tag="small")
            nc.tensor.matmul(s_ps[:], w1t_sb[:], hMt_sb[:], start=True, stop=True)

            negA = setupp.tile([O, N], BF16, tag="negA")
            Q = setupp.tile([O, N], BF16, tag="Q")
            for ch in range(2):
                sl = slice(512 * ch, 512 * (ch + 1))
                nc.vector.tensor_scalar_mul(negA[:, sl], a_ps[ch][:], -1.0)
                nc.scalar.activation(Q[:, sl], a_ps[ch][:], AF.Exp, scale=-1.0)
            s_sb = setupp.tile([O, RPC], F32, tag="s")
            nc.vector.tensor_copy(s_sb[:], s_ps[:])
            P_sb = setupp.tile([O, RPC], F32, tag="P")
            nc.scalar.activation(P_sb[:], s_ps[:], AF.Exp)

            # ---- W accumulation: W[i, j] - wbias = sum_o c2[o] (r + m) ----
            W_ps = [pbig.tile([128, 512], F32, tag="big", name=f"W_ps{ch}")
                    for ch in range(2)]
            for i in range(RPC):
                r_t = streamp.tile([O, N], BF16, tag="r")
                m_t = streamp.tile([O, N], BF16, tag="m")
                # r = relu(negA + s_i); m = min(Q * P_i, 1)
                nc.vector.tensor_scalar(
                    r_t[:], negA[:], s_sb[:, i:i + 1], 0.0, OP.add, OP.max)
                nc.vector.tensor_scalar(
                    m_t[:], Q[:], P_sb[:, i:i + 1], 1.0, OP.mult, OP.min)
                lhsT = band_sb[:, RPC - i:2 * RPC - i]
                for ch in range(2):
                    sl = slice(512 * ch, 512 * (ch + 1))
                    nc.tensor.matmul(W_ps[ch][:], lhsT, r_t[:, sl],
                                     start=(i == 0), stop=False)
                    nc.tensor.matmul(W_ps[ch][:], lhsT, m_t[:, sl],
                                     start=False, stop=(i == RPC - 1))

            # ---- W -> SBUF bf16 (+ folded conv2_b - sum(c2)) ----
            W_sb = setupp.tile([128, N], BF16, tag="W")
            for ch in range(2):
                sl = slice(512 * ch, 512 * (ch + 1))
                nc.vector.tensor_scalar(W_sb[:, sl], W_ps[ch][:], wb_sb[:],
                                        None, OP.add)

            # ---- W^T blocks, then aggT[c, i] = sum_j h[j, c] W[i, j] ----
            WT_sb = setupp.tile([128, 8, 128], BF16, tag="WT")
            for jc in range(8):
                t_ps = psmall.tile([128, 128], BF16, tag="small")
                nc.tensor.transpose(
                    t_ps[:], W_sb[:, 128 * jc:128 * (jc + 1)], eye_sb[:])
                nc.vector.tensor_copy(WT_sb[:, jc, :], t_ps[:])
            aggT_ps = psmall.tile([C, RPC], F32, tag="small")
            for jc in range(8):
                nc.tensor.matmul(aggT_ps[:], hA_bf[:, jc, :], WT_sb[:, jc, :],
                                 start=(jc == 0), stop=(jc == 7))

            # ---- out = hM + elu(agg @ lin_w.T + lin_b) ----
            aggT_ext = setupp.tile([C + 1, RPC], BF16, tag="aggT")
            nc.vector.tensor_copy(aggT_ext[0:C, :], aggT_ps[:])
            nc.vector.memset(aggT_ext[C:C + 1, :], 1.0)
            proj_ps = psmall.tile([RPC, C], F32, tag="small")
            nc.tensor.matmul(proj_ps[:], aggT_ext[:], lwt_sb[:],
                             start=True, stop=True)
            pneg = setupp.tile([RPC, C], F32, tag="pneg")
            nc.vector.tensor_scalar_min(pneg[:], proj_ps[:], 0.0)
            Ey = setupp.tile([RPC, C], F32, tag="Ey")
            nc.scalar.activation(Ey[:], pneg[:], AF.Exp)
            r2 = setupp.tile([RPC, C], F32, tag="r2")
            nc.vector.tensor_scalar_max(r2[:], proj_ps[:], 0.0)
            q2 = setupp.tile([RPC, C], F32, tag="q2")
            nc.vector.scalar_tensor_tensor(q2[:], Ey[:], -1.0, r2[:],
                                           OP.add, OP.add)
            o_sb = setupp.tile([RPC, C], F32, tag="osb")
            nc.vector.tensor_tensor(o_sb[:], q2[:], hM_sb[:], OP.add)
            nc.sync.dma_start(out[b], o_sb[:])

    nc.compile()
    return nc


def get_nc(n_repeat=1):
    if n_repeat not in _NC_CACHE:
        _NC_CACHE[n_repeat] = _build_nc(n_repeat)
    return _NC_CACHE[n_repeat]


def make_in_maps(x_pred, x_prey, x_obst, lin_w, lin_b, conv1_w, conv1_b,
                 conv2_w, conv2_b):
    f = lambda a: np.asarray(a, dtype=np.float32)
    h = np.concatenate([f(x_pred), f(x_prey), f(x_obst)], axis=1)  # [B, N, C]
    ones_n = np.ones((B, 1, N), np.float32)
    hAt = np.concatenate([h.transpose(0, 2, 1), ones_n], axis=1)   # [B, 65, N]
    c2 = f(conv2_w)[0]                                             # [O]
    band = np.zeros((O, 2 * RPC), np.float32)
    band[:, RPC] = c2
    wbias = float(f(conv2_b)[0] - c2.sum())
    wb = np.full((128, 1), wbias, np.float32)
    w1t = np.concatenate([f(conv1_w).T, f(conv1_b)[None]], axis=0)  # [65, O]
    lwt = np.concatenate([f(lin_w).T, f(lin_b)[None]], axis=0)      # [65, C]
    eye = np.eye(128, dtype=np.float32)

    in_maps = []
    for c in range(NCORES):
        r0 = c * RPC
        hm = np.ascontiguousarray(h[:, r0:r0 + RPC])
        hmt = np.concatenate(
            [hm.transpose(0, 2, 1), np.ones((B, 1, RPC), np.float32)], axis=1)
        in_maps.append({
            "hA": h, "hAt": hAt, "hM": hm, "hMt": hmt,
            "w1t": w1t, "band": band, "lwt": lwt, "eye": eye, "wb": wb,
        })
    return in_maps


def run(in_maps, trace=False, n_repeat=1):
    nc = get_nc(n_repeat)
    return run_bass_kernel_spmd(nc, in_maps, list(range(NCORES)), trace=trace)


def kernel(x_pred, x_prey, x_obst, lin_w, lin_b, conv1_w, conv1_b, conv2_w,
           conv2_b):
    in_maps = make_in_maps(x_pred, x_prey, x_obst, lin_w, lin_b, conv1_w,
                           conv1_b, conv2_w, conv2_b)
    res = run(in_maps)
    full = np.concatenate([res.results[c]["out"] for c in range(NCORES)],
                          axis=1)  # [B, N, C]
    return full[:, :NP_], full[:, NP_:NP_ + NY], full[:, NP_ + NY:]
